# revision 1
# baseline (speedup 1.0000x reference)
"""Trainium2 Bass kernel for BalancedIPRMPNN (GNN message passing).

Reference computation (G=128 disjoint graphs, NPG=512 nodes each, H=128):
    h    = x @ W_emb + b_emb
    m    = relu(GCN(h))                                  # sym-norm propagate
    virt = einsum('gnv,gnh->gvh', edge_weights, m)       # pooling (V=64)
    t1   = relu(virt @ vW1 + vb1) @ vW2 + vb2
    gf   = mean_v(t1)
    out  = relu(gf @ mW1 + mb1) @ mW2 + mb2              # [G, 10]

Key structural facts exploited (checked at runtime, numpy fallback if absent):
  * graphs are disjoint -> dense per-graph [512,512] adjacency matmul
  * edge_weights is v-uniform and nonnegative (reference uses ones/V), so all
    V virtual nodes are identical and pooling collapses to a weighted row-sum;
    the whole virtual-node MLP + mean + final MLP runs on one [H, G] tile,
    with gf's linear pair folded on the host (W23 = vW2 @ mW1).

Per graph the device does 3 tensor-engine matmuls + 2 cheap fused ops:
    P    = x_hat^T @ A_hat        (2 fp8 DoubleRow matmuls, contraction 512)
    P_sb = cast_fp16(P)           (DVE copy PSUM -> SBUF)
    M    = W1^T @ P_sb            (W1 = W_emb @ W_gcn fp16)
    s_g  = rowsum(relu(M))        (scalar activation accum_out, in-place)
where x_hat = x * dinv_src (fp8) and A_hat = (counts + I) * colw_dst (fp8),
colw = dinv * ew0 * V (FOLD_COLW=True).  With FOLD_COLW=False the adjacency
ships as exact integer counts and colw is applied on-chip via a Pool
partition-broadcast + DVE multiply (~0.3% less error, ~2% slower).  NOTE:
gpsimd must never read PSUM — it breaks on HW while passing CoreSim.

The DMA stream rides a single SP queue whose issue order is hand-chosen so
the wire never idles: the head windows pack [x g0-1 | adj g0] and
[x g8-15 | adj g7] into single transfers (one tile per DMA — the tile
framework serializes readers behind every prior writer of a tile), the last
graph's adjacency is split into two k-half tiles so its first DoubleRow
matmul starts one half-transfer earlier, and the MLP tail runs in 4-graph
chunks that hide under the main loop.

Sharding: data-parallel over graphs, 16 graphs per core on 8 cores.
"""

import ml_dtypes
import numpy as np

import concourse.mybir as mybir
import concourse.tile as tile
from concourse import bacc
from concourse.bass_utils import run_bass_kernel_spmd

# Problem constants (hardcoded per contract)
G, NPG, H, IN, OUT, V = 128, 512, 128, 128, 10, 64
N = G * NPG
N_CORES = 8
GPC = G // N_CORES          # graphs per core = 16
KB = NPG // 128             # 4 k-blocks of 128 nodes per graph
CHG = 8                     # graphs per x-chunk DMA
NCH = GPC // CHG            # x chunks per core = 2
CWG = 4                     # graphs per colw broadcast chunk

F32 = mybir.dt.float32
F16 = mybir.dt.float16
F8 = mybir.dt.float8e4

# packed tail-weight tile columns: vW1/V | W23 | mW2 | vb1 b23 mb2
TW_COLS = 2 * H + OUT + 3
C_VW1, C_W23, C_MW2 = 0, H, 2 * H
C_VB1, C_B23, C_MB2 = 2 * H + OUT, 2 * H + OUT + 1, 2 * H + OUT + 2

FOLD_COLW = True     # fold colw into the fp8 adjacency (drops the Pool
                     # broadcast + DVE multiply stage; costs ~0.3% extra err)

_CACHE = {}
_last_nc = None


def _build_program(with_bias: bool, variant=0):
    """Build the per-core Bass/Tile program (identical on all 8 cores)."""
    nc = bacc.Bacc("TRN2", target_bir_lowering=False)

    # ---- DRAM I/O ----
    # x_hat, fp8, SBUF layout, 8 graphs per chunk: [c, p, (g kb) * IN]
    xch = nc.dram_tensor("xch", [NCH, 128, CHG * KB * IN], F8, kind="ExternalInput")
    # packed windows (fewer DMAs keeps HWDGE issue ahead of the wire):
    # hx = [x g0-1 | adj g0], hx2 = [x g8-15 | adj g7]
    hx = nc.dram_tensor("hx", [128, 2 * KB * IN + KB * NPG], F8, kind="ExternalInput")
    hx2 = nc.dram_tensor("hx2", [128, CHG * KB * IN + KB * NPG], F8, kind="ExternalInput")
    # adjacency counts + I (exact small ints), 2 graphs per row: [j, p, (gg kb) * NPG]
    adjp = nc.dram_tensor("adjp", [GPC // 2, 128, 2 * KB * NPG], F8, kind="ExternalInput")
    W1 = nc.dram_tensor("W1", [IN, H], F16, kind="ExternalInput")
    TW = nc.dram_tensor("TW", [128, TW_COLS], F32, kind="ExternalInput")
    if not FOLD_COLW:
        CW = nc.dram_tensor("CW", [1, GPC * NPG], F16, kind="ExternalInput")
    if with_bias:
        biasL = nc.dram_tensor("biasL", [GPC, 2, NPG], F16, kind="ExternalInput")
        biasR = nc.dram_tensor("biasR", [2, H], F16, kind="ExternalInput")
    outT = nc.dram_tensor("outT", [OUT, GPC], F32, kind="ExternalOutput")

    DR = mybir.MatmulPerfMode.DoubleRow
    Relu = mybir.ActivationFunctionType.Relu

    with tile.TileContext(nc) as tc:
        with (
            tc.tile_pool(name="consts", bufs=1) as consts,
            tc.tile_pool(name="xp", bufs=3) as xpool,
            tc.tile_pool(name="adj", bufs=6) as apool,
            tc.tile_pool(name="psb", bufs=4) as psb_pool,
            tc.tile_pool(name="cwp", bufs=3) as cw_pool,
            tc.tile_pool(name="blp", bufs=3) as bl_pool,
            tc.tile_pool(name="pP", bufs=3, space="PSUM") as pP,
            tc.tile_pool(name="pM", bufs=3, space="PSUM") as pM,
            tc.tile_pool(name="pT", bufs=2, space="PSUM") as pT,
        ):
            # ---- input DMAs: all on the SP queue so the HWDGE issue order is
            # exactly program order.  One tile per DMA: the tile framework
            # serializes readers behind every prior writer of a tile.
            x01 = xpool.tile([128, 2 * KB * IN + KB * NPG], F8)   # x g0-1 | a g0
            x27 = xpool.tile([128, 6 * KB * IN], F8)              # graphs 2-7
            x8f = xpool.tile([128, CHG * KB * IN + KB * NPG], F8)  # x g8-15 | a g7
            adj_tiles = {}

            def dma_adj(g):
                t = apool.tile([128, KB * NPG], F8, tag="a")
                j, gg = divmod(g, 2)
                nc.sync.dma_start(
                    out=t[:], in_=adjp[j, :, gg * KB * NPG:(gg + 1) * KB * NPG])
                adj_tiles[g] = (t, 0)

            nc.sync.dma_start(out=x01[:], in_=hx[:])
            adj_tiles[0] = (x01, 2 * KB * IN)
            if not FOLD_COLW:
                CW_sb = consts.tile([1, GPC * NPG], F16)
                nc.sync.dma_start(out=CW_sb[:], in_=CW[:])
            dma_adj(1)
            W1_sb = consts.tile([IN, H], F16)
            nc.sync.dma_start(out=W1_sb[:], in_=W1[:])
            nc.sync.dma_start(out=x27[:], in_=xch[0, :, 2 * KB * IN:])
            dma_adj(2)
            dma_adj(3)
            TW_sb = consts.tile([128, TW_COLS], F32)
            nc.sync.dma_start(out=TW_sb[:], in_=TW[:])
            dma_adj(4)
            dma_adj(5)
            if with_bias:
                biasR_sb = consts.tile([2, H], F16)
                nc.sync.dma_start(out=biasR_sb[:], in_=biasR[:])

            def x_slice(g):
                if g < 2:
                    return x01, g
                if g < 8:
                    return x27, g - 2
                return x8f, g - 8

            # preload the Relu activation table while input DMAs stream
            warm = consts.tile([128, 1], F32)
            nc.gpsimd.memset(warm[:], 0.0)
            warm2 = consts.tile([128, 1], F32)
            nc.scalar.activation(out=warm2[:], in_=warm[:], func=Relu)

            s_all = consts.tile([H, GPC], F32)   # V * virt^T, one col per graph

            # colw broadcasts: all resident, issued up front on Pool
            cw_tiles = {}
            if not FOLD_COLW:
                for j in range(GPC // 2):
                    cwt = consts.tile([128, 2 * NPG], F16, name=f"cwp{j}")
                    nc.gpsimd.partition_broadcast(
                        cwt[:], CW_sb[0:1, 2 * j * NPG:(2 * j + 2) * NPG])
                    cw_tiles[j] = cwt

            # software-pipelined loop: stage A(g) = DR matmuls + colw mult;
            # stage B(g) = W1 matmul + fused relu/row-sum.
            stash = {}

            def stage_a(g):
                xt, gc = x_slice(g)
                at = adj_tiles[g]
                P_ps = pP.tile([128, NPG], F32, tag="P")
                for t in (0, 1):
                    lo = (gc * KB + 2 * t) * IN
                    lhsT = xt[:, lo:lo + 2 * IN].rearrange("p (two c) -> p two c", two=2)
                    if isinstance(at[0], tuple):
                        rhs = at[0][t][:].rearrange("p (two d) -> p two d", two=2)
                    else:
                        ro = at[1] + 2 * t * NPG
                        rhs = at[0][:, ro:ro + 2 * NPG].rearrange("p (two d) -> p two d", two=2)
                    nc.tensor.matmul(P_ps[:], lhsT, rhs,
                                     start=(t == 0), stop=(t == 1), perf_mode=DR)
                # P_sb: cast fp16; when colw is not folded into the
                # adjacency, multiply by the broadcast colw row here
                P_sb = psb_pool.tile([128, NPG], F16, tag="psb")
                if FOLD_COLW:
                    nc.vector.tensor_copy(out=P_sb[:], in_=P_ps[:])
                else:
                    cwt = cw_tiles[g // 2]
                    nc.vector.tensor_tensor(
                        out=P_sb[:], in0=P_ps[:],
                        in1=cwt[:, (g % 2) * NPG:(g % 2 + 1) * NPG],
                        op=mybir.AluOpType.mult)
                stash[g] = P_sb

            def stage_b(g):
                P_sb = stash.pop(g)
                M_ps = pM.tile([128, NPG], F32, tag="M")
                if with_bias:
                    bl = bl_pool.tile([2, NPG], F16, tag="bl")
                    nc.scalar.dma_start(out=bl[:], in_=biasL[g])
                    nc.tensor.matmul(M_ps[:], biasR_sb[:], bl[:], start=True, stop=False)
                nc.tensor.matmul(M_ps[:], W1_sb[:], P_sb[:],
                                 start=not with_bias, stop=True)
                # fused relu + row-sum; the relu'd values are only needed for
                # the sum, so write them back in place
                nc.scalar.activation(out=M_ps[:], in_=M_ps[:], func=Relu,
                                     accum_out=s_all[:, g:g + 1])

            def prefetch(g):
                # program order is after the readers of reused buffers
                if g == 0:
                    dma_adj(6)
                elif g == 1:
                    nc.sync.dma_start(out=x8f[:], in_=hx2[:])
                    adj_tiles[7] = (x8f, CHG * KB * IN)
                elif 2 <= g <= 8:
                    dma_adj(g + 6)
                elif g == 9:
                    # last graph: two k-half tiles so its first DoubleRow
                    # matmul starts one half-transfer earlier
                    j = GPC // 2 - 1
                    ta = apool.tile([128, 2 * NPG], F8, name="a15a")
                    nc.sync.dma_start(
                        out=ta[:], in_=adjp[j, :, KB * NPG:KB * NPG + 2 * NPG])
                    tb = apool.tile([128, 2 * NPG], F8, name="a15b")
                    nc.sync.dma_start(
                        out=tb[:], in_=adjp[j, :, KB * NPG + 2 * NPG:])
                    adj_tiles[15] = ((ta, tb), 0)

            # MLP tail, computed per 4-graph chunk as the s columns land
            t1 = consts.tile([H, GPC], F32)
            q1 = consts.tile([H, GPC], F32)
            o_sb = consts.tile([OUT, GPC], F32)
            mx = mybir.AluOpType.max
            add = mybir.AluOpType.add
            byp = mybir.AluOpType.bypass

            def tail_chunk(lo, hi):
                cs = slice(lo, hi)
                w = hi - lo
                pt1 = pT.tile([128, w], F32, tag="t", name="pt1")
                nc.tensor.matmul(pt1[:], TW_sb[:, C_VW1:C_VW1 + H], s_all[:, cs],
                                 start=True, stop=True)
                nc.vector.tensor_scalar(out=t1[:, cs], in0=pt1[:],
                                        scalar1=TW_sb[:, C_VB1:C_VB1 + 1],
                                        scalar2=0.0, op0=add, op1=mx)
                pt2 = pT.tile([128, w], F32, tag="t", name="pt2")
                nc.tensor.matmul(pt2[:], TW_sb[:, C_W23:C_W23 + H], t1[:, cs],
                                 start=True, stop=True)
                nc.vector.tensor_scalar(out=q1[:, cs], in0=pt2[:],
                                        scalar1=TW_sb[:, C_B23:C_B23 + 1],
                                        scalar2=0.0, op0=add, op1=mx)
                pt3 = pT.tile([OUT, w], F32, tag="t", name="pt3")
                nc.tensor.matmul(pt3[:], TW_sb[:, C_MW2:C_MW2 + OUT], q1[:, cs],
                                 start=True, stop=True)
                nc.vector.tensor_scalar(out=o_sb[:, cs], in0=pt3[:],
                                        scalar1=TW_sb[0:OUT, C_MB2:C_MB2 + 1],
                                        scalar2=0.0, op0=add, op1=byp)

            stage_a(0)
            for g in range(GPC):
                if g + 1 < GPC:
                    stage_a(g + 1)
                stage_b(g)
                prefetch(g)
                if g % 4 == 3:
                    tail_chunk(g - 3, g + 1)

            nc.sync.dma_start(out=outT[:], in_=o_sb[:])

    nc.finalize()
    return nc


def _reference_numpy(x, edge_index, W_emb, b_emb, W_gcn, b_gcn, edge_weights,
                     vW1, vb1, vW2, vb2, mW1, mb1, mW2, mb2):
    """Pure-numpy fallback (used only if structural assumptions fail)."""
    src, dst = edge_index[0].astype(np.int64), edge_index[1].astype(np.int64)
    h = x @ W_emb + b_emb
    h2 = h @ W_gcn
    deg = np.bincount(dst, minlength=N).astype(np.float32) + 1.0
    dinv = 1.0 / np.sqrt(deg)
    m = np.zeros_like(h2)
    np.add.at(m, dst, h2[src] * (dinv[src] * dinv[dst])[:, None])
    m += h2 * (dinv * dinv)[:, None]
    m = np.maximum(m + b_gcn, 0.0)
    hg = m.reshape(G, NPG, -1)
    virt = np.einsum('gnv,gnh->gvh', edge_weights, hg)
    t1 = np.maximum(virt @ vW1 + vb1, 0.0) @ vW2 + vb2
    gf = t1.mean(axis=1)
    return np.maximum(gf @ mW1 + mb1, 0.0) @ mW2 + mb2


def kernel(x, edge_index, batch, W_emb, b_emb, W_gcn, b_gcn, edge_weights,
           vW1, vb1, vW2, vb2, mW1, mb1, mW2, mb2):
    global _last_nc
    x = np.asarray(x, dtype=np.float32)
    edge_index = np.asarray(edge_index, dtype=np.int32)
    W_emb = np.asarray(W_emb, dtype=np.float32)
    b_emb = np.asarray(b_emb, dtype=np.float32)
    W_gcn = np.asarray(W_gcn, dtype=np.float32)
    b_gcn = np.asarray(b_gcn, dtype=np.float32)
    edge_weights = np.asarray(edge_weights, dtype=np.float32)
    vW1, vb1 = np.asarray(vW1, np.float32), np.asarray(vb1, np.float32)
    vW2, vb2 = np.asarray(vW2, np.float32), np.asarray(vb2, np.float32)
    mW1, mb1 = np.asarray(mW1, np.float32), np.asarray(mb1, np.float32)
    mW2, mb2 = np.asarray(mW2, np.float32), np.asarray(mb2, np.float32)

    def fallback():
        return _reference_numpy(x, edge_index, W_emb, b_emb, W_gcn, b_gcn,
                                edge_weights, vW1, vb1, vW2, vb2, mW1, mb1,
                                mW2, mb2).astype(np.float32)

    src = edge_index[0].astype(np.int64)
    dst = edge_index[1].astype(np.int64)
    if not np.array_equal(src // NPG, dst // NPG):
        return fallback()  # cross-graph edges: dense per-graph adj doesn't apply

    # pooling collapse requires v-uniform, nonnegative edge weights
    ew0 = edge_weights[:, :, 0]
    if not np.all(edge_weights == ew0[:, :, None]) or np.any(ew0 < 0):
        return fallback()

    # ---- host prep ----
    deg = (np.bincount(dst, minlength=N) + 1).astype(np.float32)
    dinv = (1.0 / np.sqrt(deg)).astype(np.float32)
    colw = (dinv * ew0.reshape(N) * np.float32(V)).astype(np.float32)  # per-dst

    # per-graph adjacency counts (+ self loops), exact small ints in fp8
    gidx = src // NPG
    lin = (gidx * NPG + (src % NPG)) * NPG + (dst % NPG)
    counts = np.bincount(lin, minlength=G * NPG * NPG)
    adjc = counts.reshape(G, NPG, NPG).astype(np.float32)  # [g, src, dst]
    diag = np.arange(NPG)
    adjc[:, diag, diag] += np.float32(1.0)
    if adjc.max() > 16.0 or np.abs(x).max() > 400.0 or colw.max() > 60000.0:
        return fallback()  # outside exact-fp8 / fp16 range
    if FOLD_COLW:
        adj_f = adjc * colw.reshape(G, 1, NPG)
        if adj_f.max() > 400.0:
            return fallback()
        adj8 = adj_f.astype(ml_dtypes.float8_e4m3)
    else:
        adj8 = adjc.astype(ml_dtypes.float8_e4m3)
    # per-graph SBUF layouts [g, p, kb*NPG + d] and [g, p, kb*IN + c]
    # SBUF layout [g, p, kb*NPG + d], then merge graph pairs
    adj_g = (adj8.reshape(G, KB, 128, NPG).transpose(0, 2, 1, 3)
             .reshape(G, 128, KB * NPG))
    adj_sb = adj_g
    adj_sb = np.ascontiguousarray(
        adj_sb.reshape(G // 2, 2, 128, KB * NPG).transpose(0, 2, 1, 3)
        .reshape(G // 2, 128, 2 * KB * NPG))
    # x_hat = dinv * x, fp8, SBUF layout [g, p, kb*IN + c], merged CHG graphs
    xs8 = (x * dinv[:, None]).astype(ml_dtypes.float8_e4m3)
    xs8g = (xs8.reshape(G, KB, 128, IN).transpose(0, 2, 1, 3)
            .reshape(G, 128, KB * IN))
    x_sb = xs8g
    x_sb = np.ascontiguousarray(
        x_sb.reshape(G // CHG, CHG, 128, KB * IN).transpose(0, 2, 1, 3)
        .reshape(G // CHG, 128, CHG * KB * IN))

    W1h = (W_emb @ W_gcn).astype(np.float16)
    W23 = (vW2 @ mW1).astype(np.float32)
    b23 = (mW1.T @ vb2 + mb1).astype(np.float32)
    TW_np = np.zeros((128, TW_COLS), np.float32)
    TW_np[:, C_VW1:C_VW1 + H] = vW1 / np.float32(V)
    TW_np[:, C_W23:C_W23 + H] = W23
    TW_np[:, C_MW2:C_MW2 + OUT] = mW2
    TW_np[:, C_VB1] = vb1
    TW_np[:, C_B23] = b23
    TW_np[:OUT, C_MB2] = mb2

    colw_g = colw.reshape(G, NPG)

    bvec = (b_emb @ W_gcn).astype(np.float32)
    with_bias = bool(np.any(bvec) or np.any(b_gcn))
    if with_bias:
        # pre-relu rank-2 correction: bvec (x) colw*wvec0 + b_gcn (x) V*ew0
        dinv_g = dinv.reshape(G, NPG)
        wvec0 = np.einsum('gsd,gs->gd', adjc, dinv_g)           # (A+I)^T dinv
        bL0 = colw_g * wvec0
        bL1 = np.float32(V) * ew0
        biasL_all = np.stack([bL0, bL1], axis=1).astype(np.float16)  # [G, 2, NPG]
        biasR_np = np.stack([bvec, b_gcn], axis=0).astype(np.float16)

    key = with_bias
    if key not in _CACHE:
        _CACHE[key] = _build_program(with_bias)
    nc = _CACHE[key]
    _last_nc = nc

    in_maps = []
    for c in range(N_CORES):
        b = c * GPC
        xg = xs8g
        im = {
            "xch": x_sb[c * NCH:(c + 1) * NCH],
            "adjp": adj_sb[c * GPC // 2:(c + 1) * GPC // 2],
            "hx": np.ascontiguousarray(np.concatenate(
                [xg[b], xg[b + 1], adj_g[b]], axis=1)),
            "hx2": np.ascontiguousarray(np.concatenate(
                [xg[b + 8 + i] for i in range(8)] + [adj_g[b + 7]], axis=1)),
            "W1": W1h,
            "TW": TW_np,
        }
        if not FOLD_COLW:
            im["CW"] = np.ascontiguousarray(
                colw_g[c * GPC:(c + 1) * GPC].reshape(1, GPC * NPG)
            ).astype(np.float16)
        if with_bias:
            im["biasL"] = np.ascontiguousarray(biasL_all[c * GPC:(c + 1) * GPC])
            im["biasR"] = biasR_np
        in_maps.append(im)

    res = run_bass_kernel_spmd(nc, in_maps, core_ids=list(range(N_CORES)))
    out = np.concatenate([res.results[c]["outT"].T for c in range(N_CORES)], axis=0)
    kernel.last_results = res
    return out.astype(np.float32)

